# revision 1
# baseline (speedup 1.0000x reference)
"""Trainium2 Bass kernel for nn_Decoder (attention decoder with 2-layer LSTM).

Contract: kernel(**inputs) takes the FULL unsharded inputs (shapes below) and
returns the full [1024, 64] output. Internally shards batch-parallel over the
8 NeuronCores, builds one SPMD Bass program (Tile framework), runs it via
run_bass_kernel_spmd, and concatenates the per-core outputs.

Per-core program design:
  - "b-layout":  [batch(128 part), feature...] for X, context, softmax.
  - "T-layout":  [feature(part), batch] for all recurrent state (hs/cs = 2*h,
                 2*c scaled states; the 0.5 factors are folded into weights on
                 the host) so PE matmuls need no per-step transposes.
  - attention score path runs in bf16 (x_projT/u/tanhU) for 2x DVE adds and
    fast PE weight loads; everything else stays fp32.
  - context (default ctx_dve path) runs on DVE in a few WIDE ops: an
    inner-transposed bf16 copy of X ([b, m, t']) makes t' innermost for both
    the E-broadcast multiply (2x DVE mode) and a contiguous reduce;
    normalization by 1/sum(E) is one reciprocal + scale at the end. Real HW
    turned out to be per-instruction/semaphore-overhead bound (~0.2-1us per
    instruction), so few wide ops beat many small matmuls (the per-t' PE
    diag-matmul variant and a DVE-score variant are kept behind flags).
  - sigmoid(x) = 0.5*(1+tanh(x/2)) everywhere so the whole kernel uses one
    ACT table set (exp_and_others: Tanh/Exp/Relu/Copy); the four gate tanhs
    fuse into one ACT op (gates reordered i,f,o,g; g-weights doubled).
  - attn_b2 is dropped (softmax shift-invariance); BatchNorm AND the fc layer
    are folded into the LSTM0 input weights on the host (W_ih0 @ fc_W'), so
    y_tilde is never materialized; LSTM biases ride ones-channel matmuls.
"""

import ml_dtypes
import numpy as np

import concourse.bass as bass
import concourse.mybir as mybir
import concourse.tile as tile
from concourse import bacc
from concourse.bass_utils import run_bass_kernel_spmd
from concourse.masks import make_identity

F32 = mybir.dt.float32
BF16 = mybir.dt.bfloat16
AF = mybir.ActivationFunctionType
OP = mybir.AluOpType

B, T, M, P, F = 1024, 50, 128, 128, 64
NCORES = 8
BL = B // NCORES  # 128 batch rows per core
BN_EPS = 1e-5

# t' chunking of the attention pipeline (u-add -> tanh -> score -> exp -> ctx).
# The last chunk is tiny so the end-of-score -> exp -> ctx -> normalize chain
# on the critical path is short.
_CW = 13
_CHUNKS = [(0, 13), (13, 13), (26, 13), (39, 9), (48, 2)]
# LSTM gates are reordered host-side to (i, f, o, g) so the three
# tanh-half-trick gates are contiguous and can fuse into one ACT op.
_GATE_PERM = (0, 1, 3, 2)
_GATE_SCALE = (0.5, 0.5, 0.5, 0.5)  # i, f, o, g (g weights are 2x)


def _bcast_mid(ap: bass.AP, n: int) -> bass.AP:
    """[p, k] AP -> [p, n, k] AP broadcast (stride 0) over the middle dim."""
    a = ap.ap
    return bass.AP(ap.tensor, ap.offset, [list(a[0]), [0, n], list(a[1])])


def _program(tc: tile.TileContext, d: dict, nsteps: int, repeat: int = 1, fused: bool = True, ctx_dve: bool = False, blay: bool = False):
    nc = tc.nc
    with (
        tc.tile_pool(name="const", bufs=1) as cp,
        tc.tile_pool(name="work", bufs=2) as wp,
        tc.tile_pool(name="dgpool", bufs=8) as dgp,
        tc.tile_pool(name="upool", bufs=5) as up,
        tc.tile_pool(name="psum", bufs=2, space="PSUM") as pp,
        tc.tile_pool(name="psum1", bufs=1, space="PSUM") as pp1,
    ):
        # ---- persistent SBUF residents -------------------------------------
        def load(name, shape, dt=F32):
            t_ = cp.tile(shape, dt, tag=name)
            nc.sync.dma_start(t_[:], d[name][:])
            return t_

        X = load("x", [BL, T, M])
        ypT = load("ypt", [F + 1, T, BL], BF16)
        w1xT = load("w1xT", [M, M])
        w1dT = load("w1dT", [P, M], BF16)
        w1cT = load("w1cT", [P, M], BF16)
        b1c = load("b1col", [M, 1])
        w2c = load("w2col", [M, 1], BF16)
        wfa = load("wfa", [M, 4 * P], BF16)
        wfb = load("wfb", [F + 1, 4 * P], BF16)
        whh0T = load("whh0T", [P, 4 * P], BF16)
        wih1T = load("wih1T", [P, 4 * P], BF16)
        whh1T = load("whh1T", [P, 4 * P], BF16)
        bias1row = load("bias1row", [1, 4 * P], BF16)
        ones_row = cp.tile([1, BL], BF16, tag="ones")
        nc.vector.memset(ones_row[:], 1.0)
        fcfh = load("fcfh", [P, F], BF16)
        fcfc = load("fcfc", [M, F], BF16)
        fcfb = load("fcfb", [F, 1])

        ident = cp.tile([128, 128], F32, tag="ident")
        make_identity(nc, ident[:])
        ident_bf = cp.tile([128, 128], BF16, tag="identbf")
        make_identity(nc, ident_bf[:])

        # bf16 copies of the attention-side tensors
        Xbf = cp.tile([BL, T, M], BF16, tag="xbf")
        nc.vector.tensor_copy(Xbf[:], X[:])
        X2 = cp.tile([BL, M, T], BF16, tag="x2")
        nc.vector.tensor_copy(X2[:], X[:].transpose([0, 2, 1]))
        if blay:
            b1r = load("b1row", [1, M])
            w2r = load("w2row", [1, M])
            onescol = cp.tile([1, 128], F32, tag="onescol")
            nc.vector.memset(onescol[:], 1.0)
            w2rp = pp.tile([128, M], F32, tag="mm")
            nc.tensor.matmul(w2rp[:], onescol[:], w2r[:], start=True, stop=True)
            w2rep = cp.tile([128, M], BF16, tag="w2rep")
            nc.scalar.copy(w2rep[:], w2rp[:])
            # xproj_b[b, t', n] = X[b,t',:] @ w1x.T + b1  (bias via k=1 matmul)
            xprojB = cp.tile([BL, T, M], BF16, tag="xprojB")
            for t_ in range(T):
                tp = pp.tile([M, BL], F32, tag="mm")
                nc.tensor.transpose(tp[:], X[:, t_, :], ident[:])
                stage = wp.tile([M, BL], F32, tag="xts")
                nc.scalar.copy(stage[:], tp[:])
                xbp = pp1.tile([BL, M], F32, tag="sc")
                nc.tensor.matmul(xbp[:], stage[:], w1xT[:], start=True, stop=False)
                nc.tensor.matmul(xbp[:], onescol[:], b1r[:], start=False, stop=True)
                nc.scalar.copy(xprojB[:, t_, :], xbp[:])
            xprojT = None
        else:
            xprojT = cp.tile([M, T, BL], BF16, tag="xprojT")

            # ---- setup: xprojT[n, t', b] = sum_m w1x[n, m] * X[b, t', m] ---
            for c0, cn in [(s, min(4, T - s)) for s in range(0, T, 4)]:
                stage = wp.tile([M, 4 * BL], F32, tag="xts")
                for k in range(cn):
                    tp = pp.tile([M, BL], F32, tag="mm")
                    nc.tensor.transpose(tp[:], X[:, c0 + k, :], ident[:])
                    nc.scalar.copy(stage[:, k * BL:(k + 1) * BL], tp[:])
                xpp = pp1.tile([M, 4 * BL], F32, tag="sc")
                nc.tensor.matmul(
                    xpp[:, : cn * BL], w1xT[:], stage[:, : cn * BL],
                    start=True, stop=True,
                )
                dst = xprojT[:, c0:c0 + cn, :].rearrange("p a b -> p (a b)")
                nc.scalar.copy(dst, xpp[:, : cn * BL])

        # ---- recurrent state (scaled: hs = 2h, cs = 2c), T-layout ----------
        # h states live in bf16 (only consumed as PE matmul operands);
        # c states stay f32 with a bf16 shadow of cs1 for the sp matmul.
        hs0 = wp.tile([P, BL], BF16, tag="hs0")
        cs0 = wp.tile([P, BL], F32, tag="cs0")
        hs1 = wp.tile([P, BL], BF16, tag="hs1")
        cs1 = wp.tile([P, BL], F32, tag="cs1")
        cs1b = wp.tile([P, BL], BF16, tag="cs1b")
        for s in (hs0, cs0, hs1, cs1, cs1b):
            nc.vector.memset(s[:], 0.0)

        ctxT = None

        def lstm_cell(mm_pairs, cs, tag):
            # gate pre-acts: g4[:, gc, :] accumulates all (lhsT, rhs) pairs.
            # Gates are (i, f, o, g) with the g-row weights doubled, so a
            # single tanh(0.5 * x) yields tanh(x/2) for i/f/o and tanh(x)
            # for g. Biases ride the ones-channel matmuls (general path).
            g4 = pp.tile([P, 4, BL], F32, tag="g4")
            for gc in range(4):
                for pi, (lh, rh) in enumerate(mm_pairs):
                    nc.tensor.matmul(g4[:, gc, :], lh[:, gc * P:(gc + 1) * P],
                                     rh, start=(pi == 0),
                                     stop=(pi == len(mm_pairs) - 1),
                                     skip_group_check=True)
            tio = wp.tile([P, 4, BL], F32, tag=f"tio{tag}")
            nc.scalar.activation(tio[:], g4[:], AF.Tanh, scale=0.5)
            ti, tf, to, tg = (tio[:, 0, :], tio[:, 1, :], tio[:, 2, :],
                              tio[:, 3, :])
            t1 = wp.tile([P, BL], F32, tag=f"t1{tag}")
            nc.vector.scalar_tensor_tensor(t1[:], tf, 1.0, cs[:], OP.add, OP.mult)
            t2 = wp.tile([P, BL], F32, tag=f"t2{tag}")
            nc.vector.scalar_tensor_tensor(t2[:], ti, 1.0, tg, OP.add, OP.mult)
            csn = wp.tile([P, BL], F32, tag=f"cs{tag}n")
            nc.vector.scalar_tensor_tensor(csn[:], t1[:], 0.5, t2[:], OP.mult, OP.add)
            tcn = wp.tile([P, BL], F32, tag=f"tc{tag}")
            nc.scalar.activation(tcn[:], csn[:], AF.Tanh, scale=0.5)
            hsn = wp.tile([P, BL], BF16, tag=f"hs{tag}n")
            nc.vector.scalar_tensor_tensor(hsn[:], to, 1.0, tcn[:], OP.add, OP.mult)
            return hsn, csn

        # ---- the T-step recurrence -----------------------------------------
        def step_body(t):
            nonlocal hs0, cs0, hs1, cs1, cs1b, ctxT
            # state projection (0.5 folds are in w1dT/w1cT):
            #  blay:  sp_b[b, n] = hs1.T@w1dT + cs1.T@w1cT   (bias is in xprojB)
            #  else:  spT[n, b] = W1d.T@hs1 + W1c.T@cs1 + b1
            spp = pp.tile([M, BL], F32, tag="mm")
            if blay:
                nc.tensor.matmul(spp[:], cs1b[:], w1cT[:], start=True, stop=False)
                nc.tensor.matmul(spp[:], hs1[:], w1dT[:], start=False, stop=True)
                sps = wp.tile([BL, M], BF16, tag="sp")
                nc.vector.tensor_copy(sps[:], spp[:])
            else:
                nc.tensor.matmul(spp[:], w1cT[:], cs1b[:], start=True, stop=False)
                nc.tensor.matmul(spp[:], w1dT[:], hs1[:], start=False, stop=True)
                sps = wp.tile([M, BL], BF16, tag="sp")
                nc.vector.tensor_scalar(sps[:], spp[:], b1c[:], None, OP.add)

            # attention + flash context accumulation, chunked over t'.
            # ctx diag-builds/matmuls for chunk c are emitted during chunk
            # c+1 so neither DVE nor PE ever stalls on the exp of the
            # current chunk (engines execute strictly in program order).
            scp = scs = ctxp = None
            if blay:
                scs = wp.tile([BL, T], F32, tag="scs")
            else:
                scp = pp1.tile([BL, T], F32, tag="sc")
            if not ctx_dve:
                ctxp = pp1.tile([BL, M], F32, tag="ctx")
            esc = wp.tile([BL, T], BF16, tag="E")
            zparts = wp.tile([BL, len(_CHUNKS)], F32, tag="Z")

            def flush_ctx(c0, cn):
                # one wide diag-batch build (single DVE instr per chunk),
                # then cn PE matmuls gated by a single semaphore
                dga = dgp.tile([128, _CW, 128], BF16, tag="dg")
                i_b = bass.AP(ident_bf[:].tensor, ident_bf[:].offset,
                              [list(ident_bf[:].ap[0]), [0, cn],
                               list(ident_bf[:].ap[1])])
                e_ap = esc[:, c0:c0 + cn]
                e_b = bass.AP(e_ap.tensor, e_ap.offset,
                              [list(e_ap.ap[0]), list(e_ap.ap[1]), [0, 128]])
                nc.vector.tensor_tensor(dga[:, :cn, :], i_b, e_b, OP.mult)
                for k in range(cn):
                    nc.tensor.matmul(ctxp[:], dga[:, k, :], Xbf[:, c0 + k, :],
                                     start=(c0 + k == 0), stop=(c0 + k == T - 1),
                                     skip_group_check=True)

            # W2 alternative: context fully on DVE in two wide mul+reduce
            # halves (t' 0:26 and 26:50), each needing only the exps of its
            # chunks; ~6 instructions replace the diag+matmul path.
            ctx_halves = []

            def flush_ctx_dve(h0, hn):
                # wm[b, m, t'] = X2 * E (t' innermost on both operands -> 2x)
                wm = wp.tile([BL, M, T // 2 + 1], BF16, tag="Wm")
                e_ap = esc[:, h0:h0 + hn]
                e_b = bass.AP(e_ap.tensor, e_ap.offset,
                              [list(e_ap.ap[0]), [0, M], list(e_ap.ap[1])])
                nc.vector.tensor_tensor(wm[:, :, :hn], X2[:, :, h0:h0 + hn],
                                        e_b, OP.mult)
                ph = wp.tile([BL, M], F32, tag=f"ctxh{len(ctx_halves)}")
                nc.vector.tensor_reduce(ph[:], wm[:, :, :hn],
                                        axis=mybir.AxisListType.X, op=OP.add)
                ctx_halves.append(ph)

            def emit_exp(ci):
                c0, cn = _CHUNKS[ci]
                src = scs if blay else scp
                nc.scalar.activation(esc[:, c0:c0 + cn], src[:, c0:c0 + cn],
                                     AF.Exp, accum_out=zparts[:, ci:ci + 1])

            # all broadcast-adds upfront so ACT's tanh chain never stalls
            us = []
            xsrc = xprojB if blay else xprojT
            for c0, cn in _CHUNKS:
                if blay:
                    u = up.tile([BL, _CW, M], BF16, tag="u")
                else:
                    u = up.tile([M, _CW, BL], BF16, tag="u")
                nc.vector.tensor_tensor(
                    u[:, :cn, :], xsrc[:, c0:c0 + cn, :],
                    _bcast_mid(sps[:], cn), OP.add)
                us.append(u)
            # tanh(c) -> score(c) -> exp(c) -> ctx flushes
            for ci, (c0, cn) in enumerate(_CHUNKS):
                if blay:
                    th = wp.tile([BL, _CW, M], BF16, tag="th")
                else:
                    th = wp.tile([M, _CW, BL], BF16, tag="th")
                nc.scalar.activation(th[:, :cn, :], us[ci][:, :cn, :], AF.Tanh)
                if blay:
                    # score[b, t'] = sum_n tanh * w2[n]: one 2x-mode multiply
                    # + one reduce per chunk on DVE (no PE matmuls at all)
                    thw = wp.tile([BL, _CW, M], BF16, tag="thw")
                    w2b = bass.AP(w2rep[:].tensor, w2rep[:].offset,
                                  [list(w2rep[:].ap[0]), [0, cn],
                                   list(w2rep[:].ap[1])])
                    nc.vector.tensor_tensor(thw[:, :cn, :], th[:, :cn, :],
                                            w2b, OP.mult)
                    nc.vector.tensor_reduce(scs[:, c0:c0 + cn], thw[:, :cn, :],
                                            axis=mybir.AxisListType.X, op=OP.add)
                else:
                    for k in range(cn):
                        nc.tensor.matmul(scp[:, c0 + k:c0 + k + 1], th[:, k, :],
                                         w2c[:], start=True, stop=True)
                emit_exp(ci)
                if ctx_dve:
                    if ci == 1:
                        flush_ctx_dve(0, 26)
                    elif ci == len(_CHUNKS) - 1:
                        flush_ctx_dve(26, 24)
                else:
                    flush_ctx(c0, cn)

            # normalize context by 1/sum(E);  [b, m] -> ctxT [m, b]
            zs = wp.tile([BL, 1], F32, tag="zs")
            nc.vector.tensor_reduce(zs[:], zparts[:],
                                    axis=mybir.AxisListType.X, op=OP.add)
            rz = wp.tile([BL, 1], F32, tag="R")
            nc.vector.reciprocal(rz[:], zs[:])
            ctx = wp.tile([BL, M], BF16, tag="ctx")
            if ctx_dve:
                csum = wp.tile([BL, M], F32, tag="csum")
                nc.vector.tensor_tensor(csum[:], ctx_halves[0][:],
                                        ctx_halves[1][:], OP.add)
                nc.vector.tensor_scalar(ctx[:], csum[:], rz[:], None, OP.mult)
            else:
                nc.vector.tensor_scalar(ctx[:], ctxp[:], rz[:], None, OP.mult)
            ctp = pp1.tile([M, BL], BF16, tag="ctp")
            nc.tensor.transpose(ctp[:], ctx[:], ident_bf[:])
            ctxT = wp.tile([M, BL], BF16, tag="ctxT")
            nc.scalar.copy(ctxT[:], ctp[:])

            # LSTM0: fc+BN are pre-folded into wfa/wfb (Wfused = W_ih0@fcW'),
            # so its gates read [ctxT; y_t; 1] and h0 directly.
            pairs0 = [(whh0T, hs0[:]), (wfa, ctxT[:]), (wfb, ypT[:, t, :])]
            hs0, cs0 = lstm_cell(pairs0, cs0, "0")
            pairs1 = [(whh1T, hs1[:]), (wih1T, hs0[:])]
            if not fused:  # nonzero LSTM1 biases ride a ones-channel matmul
                pairs1.append((bias1row, ones_row[:]))
            hs1, cs1 = lstm_cell(pairs1, cs1, "1")
            cs1b = wp.tile([P, BL], BF16, tag="cs1b")
            nc.vector.tensor_copy(cs1b[:], cs1[:])

        if repeat > 1:
            with tc.For_i(0, repeat, 1):
                for t in range(nsteps):
                    step_body(t)
        else:
            for t in range(nsteps):
                step_body(t)

        # ---- final head: relu(fcf_w @ [h1; context] + fcf_b) ---------------
        ypp = pp.tile([F, BL], F32, tag="mm")
        nc.tensor.matmul(ypp[:], fcfh[:], hs1[:], start=True, stop=False)
        nc.tensor.matmul(ypp[:], fcfc[:], ctxT[:], start=False, stop=True)
        ypre = wp.tile([F, BL], F32, tag="ypre")
        nc.scalar.activation(ypre[:], ypp[:], AF.Relu, bias=fcfb[:])
        ytp2 = pp.tile([BL, F], F32, tag="mm")
        nc.tensor.transpose(ytp2[:], ypre[:], ident[:F, :F])
        yout = wp.tile([BL, F], F32, tag="yout")
        nc.vector.tensor_copy(yout[:], ytp2[:])
        nc.sync.dma_start(d["y"][:], yout[:])


def build_program(nsteps: int = T, repeat: int = 1, fused: bool = True, ctx_dve: bool = False, blay: bool = False):
    nc = bacc.Bacc("TRN2", target_bir_lowering=False, debug=False)
    shapes = {
        "x": ([BL, T, M], F32), "ypt": ([F + 1, T, BL], BF16),
        "w1xT": ([M, M], F32),
        "w1dT": ([P, M], BF16), "w1cT": ([P, M], BF16),
        "b1col": ([M, 1], F32), "w2col": ([M, 1], BF16),
        "b1row": ([1, M], F32), "w2row": ([1, M], F32),
        "wfa": ([M, 4 * P], BF16), "wfb": ([F + 1, 4 * P], BF16),
        "whh0T": ([P, 4 * P], BF16),
        "wih1T": ([P, 4 * P], BF16), "whh1T": ([P, 4 * P], BF16),
        "bias1row": ([1, 4 * P], BF16),
        "fcfh": ([P, F], BF16), "fcfc": ([M, F], BF16), "fcfb": ([F, 1], F32),
    }
    d = {k: nc.dram_tensor(k, v[0], v[1], kind="ExternalInput") for k, v in shapes.items()}
    d["y"] = nc.dram_tensor("y", [BL, F], F32, kind="ExternalOutput")
    with tile.TileContext(nc) as tc:
        _program(tc, d, nsteps, repeat, fused, ctx_dve, blay)
    nc.compile()
    return nc


def prep_weights(inputs) -> dict:
    """Host-side layout prep of the (tiny) weight tensors, shared by all cores."""
    i = {k: np.asarray(v, dtype=np.float32) for k, v in inputs.items()}
    w1 = i["attn_w1"]
    gate_scale = np.array(_GATE_SCALE, dtype=np.float32)[None, :]

    s_eff = i["bn_gamma"] / np.sqrt(i["bn_var"] + BN_EPS)
    b_eff = i["bn_beta"] - i["bn_mean"] * s_eff
    fcw = i["fc_w"]
    fcb_row = (i["fc_b"] * s_eff + b_eff)[None, :]

    def c(a):
        return np.ascontiguousarray(a, dtype=np.float32)

    def gperm_w(wT):  # [in, 4P] -> gate blocks reordered to (i, f, o, g);
        # the g block is doubled so one tanh(0.5*x) op serves all four gates
        blocks = [wT[:, g * P:(g + 1) * P] for g in _GATE_PERM]
        blocks[3] = blocks[3] * 2.0
        return np.concatenate(blocks, 1)

    def gperm_row(b):  # [4P] -> [1, 4P] row, (i, f, o, g) with g doubled
        blocks = [b[g * P:(g + 1) * P] for g in _GATE_PERM]
        blocks[3] = blocks[3] * 2.0
        return np.concatenate(blocks)[None, :]

    # Wfused = W_ih0 @ [fc' ; fc_b'] : LSTM0 consumes [ctx; y_t; 1] directly.
    fcw_full = np.concatenate([fcw * s_eff[:, None], fcb_row.T], axis=1)  # [F, 193]
    wfused = i["w_ih0"] @ fcw_full            # [4P, 193]
    wfused[:, -1] += i["b_ih0"] + i["b_hh0"]  # LSTM0 bias on the ones channel
    wfusedT = gperm_w(wfused.T)               # [193, 4P]

    return {
        "w1dT": c(0.5 * w1[:, :P].T),
        "w1cT": c(0.5 * w1[:, P:2 * P].T),
        "w1xT": c(w1[:, 2 * P:].T),
        "b1col": c(i["attn_b1"].reshape(M, 1)),
        "w2col": c(i["attn_w2"].reshape(1, M).T),
        "b1row": c(i["attn_b1"].reshape(1, M)),
        "w2row": c(i["attn_w2"].reshape(1, M)),
        "wfa": c(wfusedT[:M]),
        "wfb": c(wfusedT[M:]),
        "whh0T": c(gperm_w(0.5 * i["w_hh0"].T)),
        "wih1T": c(gperm_w(0.5 * i["w_ih1"].T)),
        "whh1T": c(gperm_w(0.5 * i["w_hh1"].T)),
        "bias1row": c(gperm_row(i["b_ih1"] + i["b_hh1"])),
        "fcfh": c(0.5 * i["fcf_w"][:, :P].T),
        "fcfc": c(i["fcf_w"][:, P:].T),
        "fcfb": c(i["fcf_b"].reshape(F, 1)),
    }


_BF16_KEYS = ("w1dT", "w1cT", "w2col", "wfa", "wfb", "whh0T",
              "wih1T", "whh1T", "fcfh", "fcfc", "bias1row")


def make_in_maps(inputs) -> list:
    w = prep_weights(inputs)
    for k in _BF16_KEYS:
        w[k] = w[k].astype(ml_dtypes.bfloat16)
    x_all = np.asarray(inputs["X_encoded"], dtype=np.float32)
    y_all = np.asarray(inputs["y_prev"], dtype=np.float32)
    in_maps = []
    for cid in range(NCORES):
        sl = slice(cid * BL, (cid + 1) * BL)
        ypt = np.empty((F + 1, T, BL), dtype=np.float32)
        ypt[:F] = y_all[sl].transpose(2, 1, 0)
        ypt[F] = 1.0
        in_maps.append({
            "x": np.ascontiguousarray(x_all[sl]),
            "ypt": ypt.astype(ml_dtypes.bfloat16),
            **w,
        })
    return in_maps


_PROG_CACHE: dict = {}


def _get_program(nsteps: int = T, repeat: int = 1, fused: bool = True,
                 ctx_dve: bool = False, blay: bool = False):
    key = (nsteps, repeat, fused, ctx_dve, blay)
    if key not in _PROG_CACHE:
        _PROG_CACHE[key] = build_program(nsteps, repeat, fused, ctx_dve, blay)
    return _PROG_CACHE[key]


def _biases_zero(inputs) -> bool:
    return all(
        not np.any(np.asarray(inputs[k]))
        for k in ("b_ih0", "b_hh0", "b_ih1", "b_hh1")
    )


def kernel(**inputs) -> np.ndarray:
    nc = _get_program(T, fused=_biases_zero(inputs), ctx_dve=True)
    res = run_bass_kernel_spmd(nc, make_in_maps(inputs), core_ids=list(range(NCORES)))
    return np.concatenate([r["y"] for r in res.results], axis=0)

